# revision 15
# baseline (speedup 1.0000x reference)
"""Trainium2 Bass kernel for the capsule-routing layer (nn_Caps_Layer).

Full inputs: x [32, 512, 768] f32, W [1, 768, 512] f32.
Output: [32, 16, 32] f32.

Data-parallel over batch across 8 NeuronCores (4 batches/core), bf16 on
the wire. Per core, the four batches stream in sequentially and each
batch runs its complete 3-iteration routing chain as soon as its data
lands, so routing compute overlaps the DMA of later batches. The last
batch's chain is the only non-overlapped tail.

Per-batch routing (batch-local, factored so u_hat is never formed):
    m0[(nc)]  = xsum @ W           (xsum = col-sum of x)
    V[d,n]    = sum_c W[d,(n c)] * mnorm[n,c]
    b[s,n]    = x @ V              (via xT built on the PE)
    c         = softmax_n(b)
    G[d,n]    = c^T @ x
    m[(nc)]   = diag_n(W^T G)
    squash: inv = rsqrt(|m|^2) (DVE bit trick + 1 Newton step)
A tiny dependency-free PE warmup burst at t=0 pins pe_busy_start early
so the tensor engine reaches its full p-state before real work arrives.
Each batch's small PSUM outputs share one 512-col f32 bank tile.
"""
import os
import numpy as np
import concourse.bass as bass
import concourse.mybir as mybir
import concourse.tile as tile
from concourse import bacc
from concourse.bass import ts, ds
from concourse.bass_utils import run_bass_kernel_spmd

F32 = mybir.dt.float32
U32 = mybir.dt.uint32
BF16 = mybir.dt.bfloat16
AF = mybir.ActivationFunctionType
AX = mybir.AxisListType
OP = mybir.AluOpType

DEBUG = os.environ.get("KDBG", "")

NCORES = 8
B, S, D = 32, 512, 768
N, C = 16, 32
NC = N * C            # 512
BL = B // NCORES      # 4 batches per core
SCN = S // 128        # 4 s-chunks
DCN = D // 128        # 6 d-chunks
KCN = NC // 128       # 4 nc-chunks
ROUTINGS = 3
MAGIC = 0x5F3759DF

# const tile column layout (bf16)
CID = 0               # [128, 128] identity (PE transposes)
CMASK = 128           # [128, 64]  mask[(nl,c), (kc,n)] = (n == 4*kc+nl)
CONE = 192            # [128, 1]   ones
CBS = 193             # [128, 128] bsel[k, m] = (k//32 == m//32)
CMASKH = 324          # [128, 64]  0.5 * mask (folds the Newton 1/2)
CONW = 392

# column offsets inside each batch's shared psum bank tile ([128, 512] f32)
XS_O, M0_O, NSQ_O = 0, 8, 16      # nsq: +4 per routing iter
V_O, PB_O, G_O, POT_O = 32, 128, 192, 288


def _build_module():
    nc = bacc.Bacc("TRN2", target_bir_lowering=False, num_devices=NCORES)
    X = nc.dram_tensor("x", [BL, S, D], BF16, kind="ExternalInput")
    W = nc.dram_tensor("w", [D, NC], BF16, kind="ExternalInput")
    CON = nc.dram_tensor("consts", [128, CONW], BF16, kind="ExternalInput")
    OUT = nc.dram_tensor("out", [BL, N, C], F32, kind="ExternalOutput")

    with tile.TileContext(nc) as tc:
        with (
            tc.tile_pool(name="const", bufs=1) as pc,
            tc.tile_pool(name="rt", bufs=2) as prt,
            tc.tile_pool(name="pmm", bufs=3, space="PSUM") as pmm,
            tc.tile_pool(name="ptr", bufs=3, space="PSUM") as ptr,
            tc.tile_pool(name="pfin", bufs=1, space="PSUM") as pfin,
            tc.tile_pool(name="pwm", bufs=1, space="PSUM") as pwm,
        ):
            # ---- persistent tiles ----
            con = pc.tile([128, CONW], BF16, tag="con")
            wsb = pc.tile([128, DCN, NC], BF16, tag="w")
            wtsb = pc.tile([128, KCN, D], BF16, tag="wt")
            xbs = [pc.tile([128, SCN, D], BF16, tag=f"xb{b}", name=f"xb_{b}")
                   for b in range(BL)]
            xts = [pc.tile([128, DCN, S], BF16, tag=f"xt{b}", name=f"xt_{b}")
                   for b in range(BL)]
            xsumb = pc.tile([128, BL, DCN], BF16, tag="xsum")
            warm = pc.tile([128, 1], BF16, tag="warm")
            nc.vector.memset(warm[:], 1.0)
            magict = pc.tile([128, 4], U32, tag="magic")
            nc.vector.memset(magict[:], MAGIC)
            epst = pc.tile([128, 1], F32, tag="eps")
            nc.vector.memset(epst[:], 1.0)
            c3t = pc.tile([128, 4], F32, tag="c3")
            nc.vector.memset(c3t[:], 3.0)

            # ---- DMAs: consts ride the ACT queue, everything else SP ----
            nc.scalar.dma_start(con[:], CON[:, :])
            nc.sync.dma_start(
                wsb[:], W[:, :].rearrange("(dc p) n -> p dc n", p=128))
            for b in range(BL - 1):
                nc.sync.dma_start(
                    xbs[b][:], X[b, :, :].rearrange("(sc p) d -> p sc d",
                                                    p=128))
            for sc in range(SCN):
                nc.sync.dma_start(xbs[BL - 1][:, sc, :],
                                  X[BL - 1, ds(sc * 128, 128), :])

            # ---- PE warmup: dependency-free 1-col matmuls at t=0 pin
            # pe_busy_start early so everything after ~3us runs full-rate ----
            pwarm = pwm.tile([1, 1], F32, tag="warm")
            for i in range(4):
                nc.tensor.matmul(pwarm[:], warm[:], warm[:],
                                 start=True, stop=True)
            # hoist the Exp table load off the critical path
            dum = prt.tile([1, 1], F32, tag="dum")
            nc.scalar.activation(dum[:], epst[0:1, :], AF.Exp)

            ident = con[:, CID:CID + 128]
            maskr = con[:, CMASK:CMASK + KCN * N]
            maskh = con[:, CMASKH:CMASKH + KCN * N]

            dbg_done = []

            def dbg(name, ap, cols):
                # dump a [128, <=16 col] AP into OUT (flat f32) when KDBG=name
                if DEBUG != name or dbg_done:
                    return
                dbg_done.append(name)
                dt = prt.tile([128, cols], F32, tag="dbg")
                nc.vector.tensor_copy(dt[:], ap)
                flat = OUT.rearrange("b n c -> (b n c)")
                nc.sync.dma_start(
                    flat[ds(0, 128 * cols)].rearrange("(p k) -> p k", p=128),
                    dt[:])

            # ---- W transposes: wsb -> wtsb, during x0's DMA window ----
            wt_engs = (nc.vector.tensor_copy, nc.scalar.copy,
                       nc.vector.tensor_copy, nc.scalar.copy)
            for kc in range(KCN):
                ptw = ptr.tile([128, 1024], BF16, tag="tr", name=f"trw{kc}")
                for dc in range(DCN):
                    nc.tensor.transpose(
                        ptw[:, ts(dc, 128)],
                        wsb[:, dc, ds(kc * 128, 128)],
                        ident,
                    )
                wt_engs[kc](wtsb[:, kc, :], ptw[:, 0:D])

            # ---- per-batch stages ----
            xt_engs = (nc.vector.tensor_copy, nc.scalar.copy,
                       nc.vector.tensor_copy)
            pbig = {}
            trs = {}

            def stage_T(b, sc_list, first, last):
                """transposes + xsum accumulation for s-chunks of batch b."""
                xb, xt = xbs[b], xts[b]
                if first:
                    pbig[b] = pmm.tile([128, 512], F32, tag="big",
                                       name=f"big{b}")
                    trs[b] = [ptr.tile([128, 1024], BF16, tag="tr",
                                       name=f"tr{b}_{j}") for j in range(3)]
                p3 = trs[b]
                for sc in sc_list:
                    for dc in range(DCN):
                        nc.tensor.transpose(
                            p3[dc // 2][:, ds((dc % 2) * S + sc * 128, 128)],
                            xb[:, sc, ds(dc * 128, 128)],
                            ident,
                        )
                if last:
                    # xsum: keep each dc accumulation group contiguous
                    for dc in range(DCN):
                        for sc in range(SCN):
                            nc.tensor.matmul(
                                pbig[b][:, ds(XS_O + dc, 1)],
                                xb[:, sc, ds(dc * 128, 128)],
                                con[:, CONE:CONE + 1],
                                start=(sc == 0),
                                stop=(sc == SCN - 1),
                            )
                    # xsum evac first so iter0 can start immediately
                    nc.vector.tensor_copy(xsumb[:, b, :],
                                          pbig[b][:, ds(XS_O, DCN)])
                    dbg("xsum", xsumb[:, b, :], DCN)
                    for j in range(3):
                        xt_engs[j](
                            xt[:, 2 * j:2 * j + 2, :].rearrange(
                                "p a b -> p (a b)"), p3[j][:])

            def squash(pot, b, it):
                """pot: psum AP [128, (kc n)] ([128, kc] when it==0) ->
                mblk [128, (kc n)] bf16, or mnorm [128, kc] on final it."""
                final = (it == ROUTINGS - 1)
                if it == 0:
                    m = prt.tile([128, KCN], BF16, tag="m", name=f"m{b}_0")
                    nc.vector.tensor_copy(m[:], pot)
                else:
                    pm = prt.tile([128, KCN * N], BF16, tag="pm",
                                  name=f"pm{b}_{it}")
                    nc.vector.tensor_mul(pm[:], pot, maskr)
                    m = prt.tile([128, KCN], BF16, tag="m",
                                 name=f"m{b}_{it}")
                    with nc.allow_low_precision("single nonzero per group"):
                        nc.vector.tensor_reduce(
                            m[:],
                            pm[:].rearrange("p (g n) -> p g n", g=KCN),
                            axis=AX.X,
                            op=OP.add,
                        )
                sq = prt.tile([128, KCN], BF16, tag="sq", name=f"sq{b}_{it}")
                nc.gpsimd.tensor_mul(sq[:], m[:], m[:])
                pnsq = pbig[b][:, ds(NSQ_O + 4 * it, KCN)]
                nc.tensor.matmul(pnsq, con[:, CBS:CBS + 128], sq[:],
                                 start=True, stop=True)
                nsqs = prt.tile([128, KCN], F32, tag="nsqs",
                                name=f"nsqs{b}_{it}")
                nc.vector.tensor_copy(nsqs[:], pnsq)
                # rsqrt bit trick on DVE (Pool lacks TensorScalar)
                y0u = prt.tile([128, KCN], U32, tag="y0u",
                               name=f"y0u{b}_{it}")
                nc.vector.tensor_scalar(
                    y0u[:], nsqs[:].bitcast(U32), 1, None,
                    OP.logical_shift_right,
                )
                nc.vector.tensor_sub(y0u[:], magict[:], y0u[:])
                y0f = y0u[:].bitcast(F32)
                if final:
                    # full Newton on DVE
                    t1 = prt.tile([128, KCN], F32, tag="t1",
                                  name=f"t1{b}_{it}")
                    nc.vector.tensor_mul(t1[:], nsqs[:], y0f)
                    nc.vector.tensor_mul(t1[:], t1[:], y0f)
                    nc.vector.tensor_scalar(t1[:], t1[:], -0.5, 1.5,
                                            OP.mult, OP.add)
                    pinv = prt.tile([128, KCN], F32, tag="pinv",
                                    name=f"pinv{b}_{it}")
                    nc.vector.tensor_mul(pinv[:], y0f, t1[:])
                    mnorm = prt.tile([128, KCN], BF16, tag="mnf",
                                     name=f"mnf{b}")
                    nc.vector.tensor_mul(mnorm[:], m[:], pinv[:])
                    return mnorm
                # Newton on Pool via tensor-tensor only:
                # pinv2 = y0*(3 - nsq*y0^2) = 2/sqrt(nsq); the 1/2 is folded
                # into the pre-scaled mask maskh.
                t1 = prt.tile([128, KCN], F32, tag="t1", name=f"t1{b}_{it}")
                nc.gpsimd.tensor_mul(t1[:], nsqs[:], y0f)
                nc.gpsimd.tensor_mul(t1[:], t1[:], y0f)
                nc.gpsimd.tensor_sub(t1[:], c3t[:], t1[:])
                pinv = prt.tile([128, KCN], F32, tag="pinv",
                                name=f"pinv{b}_{it}")
                nc.gpsimd.tensor_mul(pinv[:], y0f, t1[:])
                # scatter m over the half-scaled diag mask, fold pinv2 in
                mm1 = prt.tile([128, KCN * N], BF16, tag="mm1",
                               name=f"mm1_{b}_{it}")
                m_bc = bass.AP(
                    tensor=m.tensor, offset=m.offset,
                    ap=[m.ap[0], [1, KCN], [0, N]],
                )
                nc.gpsimd.tensor_mul(
                    mm1[:].rearrange("p (g n) -> p g n", g=KCN),
                    m_bc,
                    maskh.rearrange("p (g n) -> p g n", g=KCN),
                )
                mblk = prt.tile([128, KCN * N], BF16, tag="mblk",
                                name=f"mblk{b}_{it}")
                pi_bc = bass.AP(
                    tensor=pinv.tensor, offset=pinv.offset,
                    ap=[pinv.ap[0], [1, KCN], [0, N]],
                )
                nc.gpsimd.tensor_mul(
                    mblk[:].rearrange("p (g n) -> p g n", g=KCN),
                    mm1[:].rearrange("p (g n) -> p g n", g=KCN),
                    pi_bc,
                )
                return mblk

            def v_and_b(mblk, b, it):
                """V = W @ diag(mblk) then b-logits = x @ V -> psum."""
                for dc in range(DCN):
                    for kc in range(KCN):
                        nc.tensor.matmul(
                            pbig[b][:, ds(V_O + dc * N, N)],
                            wtsb[:, kc, ds(dc * 128, 128)],
                            mblk[:, ds(kc * N, N)],
                            start=(kc == 0),
                            stop=(kc == KCN - 1),
                        )
                vsb = prt.tile([128, DCN * N], BF16, tag="vsb",
                               name=f"vsb{b}_{it}")
                (nc.scalar.copy if it == 0 else
                 nc.vector.tensor_copy)(vsb[:], pbig[b][:, ds(V_O, DCN * N)])
                dbg(f"vsb{it}", vsb[:, 0:16], 16)
                for sc in range(SCN):
                    for dc in range(DCN):
                        nc.tensor.matmul(
                            pbig[b][:, ds(PB_O + sc * N, N)],
                            xts[b][:, dc, ds(sc * 128, 128)],
                            vsb[:, ds(dc * N, N)],
                            start=(dc == 0),
                            stop=(dc == DCN - 1),
                        )
                dbg(f"pb{it}", pbig[b][:, ds(PB_O, 16)], 16)
                return pbig[b][:, ds(PB_O, SCN * N)]

            def iter0(b):
                for kc in range(KCN):
                    for dc in range(DCN):
                        nc.tensor.matmul(
                            pbig[b][:, ds(M0_O + kc, 1)],
                            wsb[:, dc, ds(kc * 128, 128)],
                            xsumb[:, b, ds(dc, 1)],
                            start=(dc == 0),
                            stop=(dc == DCN - 1),
                        )
                dbg("m0", pbig[b][:, ds(M0_O, KCN)], KCN)
                mblk = squash(pbig[b][:, ds(M0_O, KCN)], b, 0)
                dbg("mblk0", mblk[:, 0:16], 16)
                return v_and_b(mblk, b, 0)

            def softmax_G_pot(pb, b, it):
                expb = prt.tile([128, SCN * N], BF16, tag="expb",
                                name=f"expb{b}_{it}")
                nc.scalar.activation(expb[:], pb, AF.Exp)
                zsum = prt.tile([128, SCN], BF16, tag="zsum",
                                name=f"zsum{b}_{it}")
                with nc.allow_low_precision("softmax denom; scale cancels"):
                    nc.vector.tensor_reduce(
                        zsum[:],
                        expb[:].rearrange("p (g n) -> p g n", g=SCN),
                        axis=AX.X,
                        op=OP.add,
                    )
                zrec = prt.tile([128, SCN], F32, tag="zrec",
                                name=f"zrec{b}_{it}")
                nc.vector.reciprocal(zrec[:], zsum[:])
                cw = prt.tile([128, SCN * N], BF16, tag="cw",
                              name=f"cw{b}_{it}")
                zr_bc = bass.AP(
                    tensor=zrec.tensor, offset=zrec.offset,
                    ap=[zrec.ap[0], [1, SCN], [0, N]],
                )
                nc.gpsimd.tensor_mul(
                    cw[:].rearrange("p (g n) -> p g n", g=SCN),
                    expb[:].rearrange("p (g n) -> p g n", g=SCN),
                    zr_bc,
                )
                for dc in range(DCN):
                    for sc in range(SCN):
                        nc.tensor.matmul(
                            pbig[b][:, ds(G_O + dc * N, N)],
                            xbs[b][:, sc, ds(dc * 128, 128)],
                            cw[:, ds(sc * N, N)],
                            start=(sc == 0),
                            stop=(sc == SCN - 1),
                        )
                dbg(f"cw{it}", cw[:, 0:16], 16)
                gsb = prt.tile([128, DCN * N], BF16, tag="gsb",
                               name=f"gsb{b}_{it}")
                (nc.scalar.copy if it == 1 else
                 nc.vector.tensor_copy)(gsb[:], pbig[b][:, ds(G_O, DCN * N)])
                dbg(f"gsb{it}", gsb[:, 0:16], 16)
                for kc in range(KCN):
                    for dc in range(DCN):
                        nc.tensor.matmul(
                            pbig[b][:, ds(POT_O + kc * N, N)],
                            wsb[:, dc, ds(kc * 128, 128)],
                            gsb[:, ds(dc * N, N)],
                            start=(dc == 0),
                            stop=(dc == DCN - 1),
                        )
                return pbig[b][:, ds(POT_O, KCN * N)]

            def stage_S(pb, b, it):
                pot = softmax_G_pot(pb, b, it)
                dbg(f"pot{it}", pbig[b][:, ds(POT_O, 16)], 16)
                r = squash(pot, b, it)
                if it == ROUTINGS - 1:
                    dbg("mnorm", r[:], KCN)
                if it < ROUTINGS - 1:
                    return v_and_b(r, b, it)
                # final: transpose to [(kc), (nl c)] and DMA out
                pfin_t = pfin.tile([KCN, 128], BF16, tag="fin",
                                   name=f"fin{b}")
                nc.tensor.transpose(pfin_t[:], r[:], ident)
                fsb = prt.tile([KCN, 128], F32, tag="fsb", name=f"fsb{b}")
                nc.vector.tensor_copy(fsb[:], pfin_t[:])
                if not DEBUG:
                    nc.sync.dma_start(
                        OUT[b].rearrange("(kc nl) c -> kc (nl c)", nl=4),
                        fsb[:],
                    )
                return None

            # ---- pipeline: emission order approximates readiness order ----
            pbs = {}
            stage_T(0, range(SCN), True, True)           # T0
            pbs[0] = iter0(0)                            # I0(0)
            pbs[0] = stage_S(pbs[0], 0, 1)               # S1(0)
            stage_T(1, range(SCN), True, True)           # T1
            stage_S(pbs[0], 0, 2)                        # S2(0)
            pbs[1] = iter0(1)                            # I0(1)
            pbs[1] = stage_S(pbs[1], 1, 1)               # S1(1)
            stage_T(2, range(SCN), True, True)           # T2
            pbs[2] = iter0(2)                            # I0(2)
            stage_S(pbs[1], 1, 2)                        # S2(1)
            stage_T(3, range(3), True, False)            # T3 c0-2
            pbs[2] = stage_S(pbs[2], 2, 1)               # S1(2)
            stage_T(3, range(3, SCN), False, True)       # T3 c3
            pbs[3] = iter0(3)                            # I0(3)
            stage_S(pbs[2], 2, 2)                        # S2(2)
            pbs[3] = stage_S(pbs[3], 3, 1)               # S1(3)
            stage_S(pbs[3], 3, 2)                        # S2(3)

    nc.compile()
    return nc


def _make_consts():
    import ml_dtypes
    con = np.zeros((128, CONW), dtype=np.float32)
    con[:, CID:CID + 128] = np.eye(128, dtype=np.float32)
    p = np.arange(128)
    for kc in range(KCN):
        for n in range(N):
            con[:, CMASK + kc * N + n] = (n == 4 * kc + p // 32)
    con[:, CONE] = 1.0
    con[:, CBS:CBS + 128] = (p[:, None] // 32 == p[None, :] // 32)
    con[:, CMASKH:CMASKH + KCN * N] = 0.5 * con[:, CMASK:CMASK + KCN * N]
    return con.astype(ml_dtypes.bfloat16)


_NC_CACHE = []


def kernel(x: np.ndarray, W: np.ndarray) -> np.ndarray:
    import ml_dtypes
    assert x.shape == (B, S, D) and W.shape == (1, D, NC)
    if not _NC_CACHE:
        _NC_CACHE.append(_build_module())
    nc = _NC_CACHE[0]
    con = _make_consts()
    w2 = np.ascontiguousarray(W[0]).astype(ml_dtypes.bfloat16)
    xb = x.astype(ml_dtypes.bfloat16)
    in_maps = []
    for i in range(NCORES):
        m = {
            "x": np.ascontiguousarray(xb[i * BL:(i + 1) * BL]),
            "w": w2,
            "consts": con,
        }
        in_maps.append(m)
    res = run_bass_kernel_spmd(nc, in_maps, list(range(NCORES)))
    out = np.concatenate([res.results[i]["out"] for i in range(NCORES)],
                         axis=0)
    return out.astype(np.float32)


# revision 18
# speedup vs baseline: 1.2200x; 1.2200x over previous
"""Trainium2 Bass kernel for the capsule-routing layer (nn_Caps_Layer).

Full inputs: x [32, 512, 768] f32, W [1, 768, 512] f32.
Output: [32, 16, 32] f32.

Data-parallel over batch across 8 NeuronCores (4 batches/core), bf16 on
the wire. Per core the four batches stream in sequentially; batches are
processed as two PAIRS, each pair running its complete 3-iteration
routing chain as soon as both batches land, so pair-0's routing overlaps
pair-1's DMA. Elementwise work is pair-wide (halves op count and
sequencer pressure); matmul accumulation groups stay per-batch and
contiguous. Bulk work (transposes, PSUM evacuations) is deprioritized
for the tile scheduler so it fills engine gaps instead of delaying the
latency-critical routing chain.

Per-batch routing math (batch-local, u_hat never materialized):
    m0[(nc)]  = xsum @ W           (xsum = col-sum of x)
    V[d,n]    = sum_c W[d,(n c)] * mnorm[n,c]
    b[s,n]    = x @ V              (via xT built on the PE)
    c         = softmax_n(b)
    G[d,n]    = c^T @ x
    m[(nc)]   = diag_n(W^T G)
    squash: inv = rsqrt(|m|^2) (bit trick + 1 Newton step on DVE)
A dependency-free PE warmup burst at t=0 pins pe_busy_start so the
tensor engine reaches full p-state before real work arrives.
"""
import os
import numpy as np
import concourse.bass as bass
import concourse.mybir as mybir
import concourse.tile as tile
from concourse import bacc
from concourse.bass import ts, ds
from concourse.bass_utils import run_bass_kernel_spmd

F32 = mybir.dt.float32
U32 = mybir.dt.uint32
BF16 = mybir.dt.bfloat16
AF = mybir.ActivationFunctionType
AX = mybir.AxisListType
OP = mybir.AluOpType

DEBUG = os.environ.get("KDBG", "")

NCORES = 8
B, S, D = 32, 512, 768
N, C = 16, 32
NC = N * C            # 512
BL = B // NCORES      # 4 batches per core
SCN = S // 128        # 4 s-chunks
DCN = D // 128        # 6 d-chunks
KCN = NC // 128       # 4 nc-chunks
ROUTINGS = 3
MAGIC = 0x5F3759DF
BULK = -1 << 20       # scheduler deprioritization offset for bulk work

# const tile column layout (bf16)
CID = 0               # [128, 128] identity (PE transposes)
CMASK = 128           # [128, 64]  mask[(nl,c), (kc,n)] = (n == 4*kc+nl)
CONE = 192            # [128, 1]   ones
CBS = 193             # [128, 128] bsel[k, m] = (k//32 == m//32)
CONW = 324

# column offsets inside each pair's shared psum bank tile ([128, 512] f32)
# t = batch parity within the pair
XS_O = 0              # + 6*t   (6 cols each)
M0_O = 16             # + 4*t
NSQ_O = 24            # + 8*it  ([128, 8] per squash)
VG_O = 64             # + 96*t  (V, later reused by G)
PBP_O = 256           # + 64*t  (b-logits, later reused by pot)


def _build_module():
    nc = bacc.Bacc("TRN2", target_bir_lowering=False, num_devices=NCORES)
    X = nc.dram_tensor("x", [BL, S, D], BF16, kind="ExternalInput")
    W = nc.dram_tensor("w", [D, NC], BF16, kind="ExternalInput")
    CON = nc.dram_tensor("consts", [128, CONW], BF16, kind="ExternalInput")
    OUT = nc.dram_tensor("out", [BL, N, C], F32, kind="ExternalOutput")

    with tile.TileContext(nc) as tc:
        with (
            tc.tile_pool(name="const", bufs=1) as pc,
            tc.tile_pool(name="rt", bufs=2) as prt,
            tc.tile_pool(name="pmm", bufs=2, space="PSUM") as pmm,
            tc.tile_pool(name="ptr", bufs=4, space="PSUM") as ptr,
            tc.tile_pool(name="pfin", bufs=1, space="PSUM") as pfin,
            tc.tile_pool(name="pwm", bufs=1, space="PSUM") as pwm,
        ):
            # ---- persistent tiles ----
            con = pc.tile([128, CONW], BF16, tag="con")
            wsb = pc.tile([128, DCN, NC], BF16, tag="w")
            wtsb = pc.tile([128, KCN, D], BF16, tag="wt")
            xbs = [pc.tile([128, SCN, D], BF16, tag=f"xb{b}", name=f"xb_{b}")
                   for b in range(BL)]
            xts = [pc.tile([128, DCN, S], BF16, tag=f"xt{b}", name=f"xt_{b}")
                   for b in range(BL)]
            xsumb = pc.tile([128, BL, DCN], BF16, tag="xsum")
            warm = pc.tile([128, 1], BF16, tag="warm")
            nc.vector.memset(warm[:], 1.0)
            magict = pc.tile([128, 8], U32, tag="magic")
            nc.vector.memset(magict[:], MAGIC)
            epst = pc.tile([128, 1], F32, tag="eps")
            nc.vector.memset(epst[:], 1.0)

            # ---- DMAs: consts ride the ACT queue, everything else SP ----
            nc.scalar.dma_start(con[:], CON[:, :])
            nc.sync.dma_start(
                wsb[:], W[:, :].rearrange("(dc p) n -> p dc n", p=128))
            for b in range(BL - 1):
                nc.sync.dma_start(
                    xbs[b][:], X[b, :, :].rearrange("(sc p) d -> p sc d",
                                                    p=128))
            for sc in range(SCN):
                nc.sync.dma_start(xbs[BL - 1][:, sc, :],
                                  X[BL - 1, ds(sc * 128, 128), :])

            # ---- PE warmup + Exp table prefetch ----
            pwarm = pwm.tile([1, 1], F32, tag="warm")
            for i in range(4):
                nc.tensor.matmul(pwarm[:], warm[:], warm[:],
                                 start=True, stop=True)
            dum = prt.tile([1, 1], F32, tag="dum")
            nc.scalar.activation(dum[:], epst[0:1, :], AF.Exp)

            ident = con[:, CID:CID + 128]

            def mask_bc(base, tcount, gcount):
                """con mask [128, 64] broadcast to [128, tcount, gcount*N]."""
                src = con[:, base:base + KCN * N]
                return bass.AP(
                    tensor=src.tensor, offset=src.offset,
                    ap=[src.ap[0], [0, tcount], [1, KCN * N]],
                )

            dbg_done = []

            def dbg(name, ap, cols):
                if DEBUG != name or dbg_done:
                    return
                dbg_done.append(name)
                dt = prt.tile([128, cols], F32, tag="dbg")
                nc.vector.tensor_copy(dt[:], ap)
                flat = OUT.rearrange("b n c -> (b n c)")
                nc.sync.dma_start(
                    flat[ds(0, 128 * cols)].rearrange("(p k) -> p k", p=128),
                    dt[:])

            # ---- W transposes (bulk) ----
            with tc.high_priority(BULK):
                wt_engs = (nc.vector.tensor_copy, nc.scalar.copy,
                           nc.vector.tensor_copy, nc.scalar.copy)
                for kc in range(KCN):
                    ptw = ptr.tile([128, 1024], BF16, tag="tr",
                                   name=f"trw{kc}")
                    for dc in range(DCN):
                        nc.tensor.transpose(
                            ptw[:, ts(dc, 128)],
                            wsb[:, dc, ds(kc * 128, 128)],
                            ident,
                        )
                    wt_engs[kc](wtsb[:, kc, :], ptw[:, 0:D])

            # ---- per-batch transposes + xsum (bulk) ----
            pbig = {}
            trs = {}

            def stage_T(b, sc_list, first, last):
                xb, xt = xbs[b], xts[b]
                t = b % 2
                if first and t == 0:
                    pbig[b // 2] = pmm.tile([128, 512], F32, tag="big",
                                            name=f"big{b // 2}")
                if first:
                    trs[b] = [ptr.tile([128, 1024], BF16, tag="tr",
                                       name=f"tr{b}_{j}") for j in range(3)]
                pg = pbig[b // 2]
                with tc.high_priority(BULK):
                    p3 = trs[b]
                    for sc in sc_list:
                        for dc in range(DCN):
                            nc.tensor.transpose(
                                p3[dc // 2][:, ds((dc % 2) * S + sc * 128,
                                                  128)],
                                xb[:, sc, ds(dc * 128, 128)],
                                ident,
                            )
                    if last:
                        for dc in range(DCN):
                            for sc in range(SCN):
                                nc.tensor.matmul(
                                    pg[:, ds(XS_O + 6 * t + dc, 1)],
                                    xb[:, sc, ds(dc * 128, 128)],
                                    con[:, CONE:CONE + 1],
                                    start=(sc == 0),
                                    stop=(sc == SCN - 1),
                                )
                        nc.vector.tensor_copy(xsumb[:, b, :],
                                              pg[:, ds(XS_O + 6 * t, DCN)])
                        for j, eng in enumerate(
                                (nc.vector.tensor_copy, nc.scalar.copy,
                                 nc.vector.tensor_copy if b % 2 else
                                 nc.scalar.copy)):
                            eng(xt[:, 2 * j:2 * j + 2, :].rearrange(
                                "p a b -> p (a b)"), p3[j][:])

            # ---- pair-wide routing chain ----
            def squash(pot, p, it):
                """pot: psum AP [128, (t kc n)] ([128, (t kc)] when it==0)
                -> mblk [128, (t kc n)] bf16 / mnorm [128, (t kc)] final."""
                final = (it == ROUTINGS - 1)
                W8 = 2 * KCN
                if it == 0:
                    m = prt.tile([128, W8], BF16, tag="m", name=f"m{p}_0")
                    nc.vector.tensor_copy(m[:], pot)
                else:
                    pm = prt.tile([128, 2 * KCN * N], BF16, tag="pm",
                                  name=f"pm{p}_{it}")
                    nc.vector.tensor_mul(
                        pm[:].rearrange("p (t k) -> p t k", t=2),
                        pot.rearrange("p (t k) -> p t k", t=2),
                        mask_bc(CMASK, 2, KCN),
                    )
                    m = prt.tile([128, W8], BF16, tag="m", name=f"m{p}_{it}")
                    with nc.allow_low_precision("single nonzero per group"):
                        nc.vector.tensor_reduce(
                            m[:],
                            pm[:].rearrange("p (g n) -> p g n", g=W8),
                            axis=AX.X,
                            op=OP.add,
                        )
                sq = prt.tile([128, W8], BF16, tag="sq", name=f"sq{p}_{it}")
                nc.gpsimd.tensor_mul(sq[:], m[:], m[:])
                pnsq = pbig[p][:, ds(NSQ_O + 8 * it, W8)]
                nc.tensor.matmul(pnsq, con[:, CBS:CBS + 128], sq[:],
                                 start=True, stop=True)
                # rsqrt: bit trick + 1 Newton step on DVE (reads PSUM)
                y0u = prt.tile([128, W8], U32, tag="y0u",
                               name=f"y0u{p}_{it}")
                nc.vector.tensor_scalar(
                    y0u[:], pnsq.bitcast(U32), 1, None,
                    OP.logical_shift_right,
                )
                nc.vector.tensor_sub(y0u[:], magict[:], y0u[:])
                y0f = y0u[:].bitcast(F32)
                t1 = prt.tile([128, W8], F32, tag="t1", name=f"t1{p}_{it}")
                nc.vector.tensor_mul(t1[:], pnsq, y0f)
                nc.vector.tensor_mul(t1[:], t1[:], y0f)
                nc.vector.tensor_scalar(t1[:], t1[:], -0.5, 1.5,
                                        OP.mult, OP.add)
                pinv = prt.tile([128, W8], F32, tag="pinv",
                                name=f"pinv{p}_{it}")
                nc.vector.tensor_mul(pinv[:], y0f, t1[:])
                if final:
                    mnorm = prt.tile([128, W8], BF16, tag="mnf",
                                     name=f"mnf{p}")
                    nc.vector.tensor_mul(mnorm[:], m[:], pinv[:])
                    return mnorm
                # scatter m over the diag mask on Pool (overlaps DVE Newton)
                mm1 = prt.tile([128, 2 * KCN * N], BF16, tag="mm1",
                               name=f"mm1_{p}_{it}")
                msrc = con[:, CMASK:CMASK + KCN * N]
                nc.gpsimd.tensor_mul(
                    mm1[:].rearrange("p (t g n) -> p t g n", t=2, g=KCN),
                    bass.AP(tensor=m.tensor, offset=m.offset,
                            ap=[m.ap[0], [KCN, 2], [1, KCN], [0, N]]),
                    bass.AP(tensor=msrc.tensor, offset=msrc.offset,
                            ap=[msrc.ap[0], [0, 2], [N, KCN], [1, N]]),
                )
                mblk = prt.tile([128, 2 * KCN * N], BF16, tag="mblk",
                                name=f"mblk{p}_{it}")
                pi_bc = bass.AP(
                    tensor=pinv.tensor, offset=pinv.offset,
                    ap=[pinv.ap[0], [1, W8], [0, N]],
                )
                nc.gpsimd.tensor_mul(
                    mblk[:].rearrange("p (g n) -> p g n", g=W8),
                    mm1[:].rearrange("p (g n) -> p g n", g=W8),
                    pi_bc,
                )
                return mblk

            def v_and_b(mblk, p, it):
                pg = pbig[p]
                for t in range(2):
                    for dc in range(DCN):
                        for kc in range(KCN):
                            nc.tensor.matmul(
                                pg[:, ds(VG_O + 96 * t + dc * N, N)],
                                wtsb[:, kc, ds(dc * 128, 128)],
                                mblk[:, ds(t * KCN * N + kc * N, N)],
                                start=(kc == 0),
                                stop=(kc == KCN - 1),
                            )
                vsb = prt.tile([128, 2 * DCN * N], BF16, tag="vsb",
                               name=f"vsb{p}_{it}")
                (nc.scalar.copy if it == 0 else
                 nc.vector.tensor_copy)(vsb[:], pg[:, ds(VG_O, 2 * DCN * N)])
                dbg(f"vsb{it}", vsb[:, 0:16], 16)
                for t in range(2):
                    for sc in range(SCN):
                        for dc in range(DCN):
                            nc.tensor.matmul(
                                pg[:, ds(PBP_O + 64 * t + sc * N, N)],
                                xts[2 * p + t][:, dc, ds(sc * 128, 128)],
                                vsb[:, ds(96 * t + dc * N, N)],
                                start=(dc == 0),
                                stop=(dc == DCN - 1),
                            )
                dbg(f"pb{it}", pg[:, ds(PBP_O, 16)], 16)
                return pg[:, ds(PBP_O, 2 * SCN * N)]

            def iter0(p):
                pg = pbig[p]
                for t in range(2):
                    for kc in range(KCN):
                        for dc in range(DCN):
                            nc.tensor.matmul(
                                pg[:, ds(M0_O + 4 * t + kc, 1)],
                                wsb[:, dc, ds(kc * 128, 128)],
                                xsumb[:, 2 * p + t, ds(dc, 1)],
                                start=(dc == 0),
                                stop=(dc == DCN - 1),
                            )
                dbg("m0", pg[:, ds(M0_O, 8)], 8)
                mblk = squash(pg[:, ds(M0_O, 2 * KCN)], p, 0)
                dbg("mblk0", mblk[:, 0:16], 16)
                return v_and_b(mblk, p, 0)

            def softmax_G_pot(pb, p, it):
                pg = pbig[p]
                W128 = 2 * SCN * N
                expb = prt.tile([128, W128], BF16, tag="expb",
                                name=f"expb{p}_{it}")
                nc.scalar.activation(expb[:], pb, AF.Exp)
                zsum = prt.tile([128, 2 * SCN], BF16, tag="zsum",
                                name=f"zsum{p}_{it}")
                with nc.allow_low_precision("softmax denom; scale cancels"):
                    nc.vector.tensor_reduce(
                        zsum[:],
                        expb[:].rearrange("p (g n) -> p g n", g=2 * SCN),
                        axis=AX.X,
                        op=OP.add,
                    )
                zrec = prt.tile([128, 2 * SCN], F32, tag="zrec",
                                name=f"zrec{p}_{it}")
                nc.vector.reciprocal(zrec[:], zsum[:])
                cw = prt.tile([128, W128], BF16, tag="cw",
                              name=f"cw{p}_{it}")
                zr_bc = bass.AP(
                    tensor=zrec.tensor, offset=zrec.offset,
                    ap=[zrec.ap[0], [1, 2 * SCN], [0, N]],
                )
                nc.gpsimd.tensor_mul(
                    cw[:].rearrange("p (g n) -> p g n", g=2 * SCN),
                    expb[:].rearrange("p (g n) -> p g n", g=2 * SCN),
                    zr_bc,
                )
                dbg(f"cw{it}", cw[:, 0:16], 16)
                for t in range(2):
                    for dc in range(DCN):
                        for sc in range(SCN):
                            nc.tensor.matmul(
                                pg[:, ds(VG_O + 96 * t + dc * N, N)],
                                xbs[2 * p + t][:, sc, ds(dc * 128, 128)],
                                cw[:, ds(64 * t + sc * N, N)],
                                start=(sc == 0),
                                stop=(sc == SCN - 1),
                            )
                gsb = prt.tile([128, 2 * DCN * N], BF16, tag="gsb",
                               name=f"gsb{p}_{it}")
                (nc.scalar.copy if it == 1 else
                 nc.vector.tensor_copy)(gsb[:], pg[:, ds(VG_O, 2 * DCN * N)])
                dbg(f"gsb{it}", gsb[:, 0:16], 16)
                for t in range(2):
                    for kc in range(KCN):
                        for dc in range(DCN):
                            nc.tensor.matmul(
                                pg[:, ds(PBP_O + 64 * t + kc * N, N)],
                                wsb[:, dc, ds(kc * 128, 128)],
                                gsb[:, ds(96 * t + dc * N, N)],
                                start=(dc == 0),
                                stop=(dc == DCN - 1),
                            )
                return pg[:, ds(PBP_O, 2 * KCN * N)]

            def stage_S(pb, p, it):
                pot = softmax_G_pot(pb, p, it)
                dbg(f"pot{it}", pbig[p][:, ds(PBP_O, 16)], 16)
                r = squash(pot, p, it)
                if it == ROUTINGS - 1:
                    dbg("mnorm", r[:, 0:8], 8)
                if it < ROUTINGS - 1:
                    return v_and_b(r, p, it)
                pfin_t = pfin.tile([2 * KCN, 128], BF16, tag="fin",
                                   name=f"fin{p}")
                nc.tensor.transpose(pfin_t[:], r[:], ident)
                fsb = prt.tile([2 * KCN, 128], F32, tag="fsb",
                               name=f"fsb{p}")
                nc.vector.tensor_copy(fsb[:], pfin_t[:])
                if not DEBUG:
                    nc.sync.dma_start(
                        OUT[2 * p:2 * p + 2].rearrange(
                            "b (kc nl) c -> (b kc) (nl c)", nl=4),
                        fsb[:],
                    )
                return None

            # ---- pipeline ----
            pbs = {}
            stage_T(0, range(SCN), True, True)           # T0
            stage_T(1, range(SCN), True, True)           # T1
            pbs[0] = iter0(0)                            # I0(P0)
            pbs[0] = stage_S(pbs[0], 0, 1)               # S1(P0)
            stage_T(2, range(SCN), True, True)           # T2
            stage_S(pbs[0], 0, 2)                        # S2(P0)
            stage_T(3, range(3), True, False)            # T3 c0-2
            stage_T(3, range(3, SCN), False, True)       # T3 c3
            pbs[1] = iter0(1)                            # I0(P1)
            pbs[1] = stage_S(pbs[1], 1, 1)               # S1(P1)
            stage_S(pbs[1], 1, 2)                        # S2(P1)

    nc.compile()
    return nc


def _make_consts():
    import ml_dtypes
    con = np.zeros((128, CONW), dtype=np.float32)
    con[:, CID:CID + 128] = np.eye(128, dtype=np.float32)
    p = np.arange(128)
    for kc in range(KCN):
        for n in range(N):
            con[:, CMASK + kc * N + n] = (n == 4 * kc + p // 32)
    con[:, CONE] = 1.0
    con[:, CBS:CBS + 128] = (p[:, None] // 32 == p[None, :] // 32)
    return con.astype(ml_dtypes.bfloat16)


_NC_CACHE = []


def kernel(x: np.ndarray, W: np.ndarray) -> np.ndarray:
    import ml_dtypes
    assert x.shape == (B, S, D) and W.shape == (1, D, NC)
    if not _NC_CACHE:
        _NC_CACHE.append(_build_module())
    nc = _NC_CACHE[0]
    con = _make_consts()
    w2 = np.ascontiguousarray(W[0]).astype(ml_dtypes.bfloat16)
    xb = x.astype(ml_dtypes.bfloat16)
    in_maps = []
    for i in range(NCORES):
        m = {
            "x": np.ascontiguousarray(xb[i * BL:(i + 1) * BL]),
            "w": w2,
            "consts": con,
        }
        in_maps.append(m)
    res = run_bass_kernel_spmd(nc, in_maps, list(range(NCORES)))
    out = np.concatenate([res.results[i]["out"] for i in range(NCORES)],
                         axis=0)
    return out.astype(np.float32)


# revision 23
# speedup vs baseline: 1.6126x; 1.3218x over previous
"""Trainium2 Bass kernel for the capsule-routing layer (nn_Caps_Layer).

Full inputs: x [32, 512, 768] f32, W [1, 768, 512] f32.
Output: [32, 16, 32] f32.

Strategy: data-parallel over batch across 8 NeuronCores (4 batches/core),
inputs converted to bf16 on the host (halves the HBM traffic; rel-err
budget 2e-2 >> bf16's ~5e-3).

Per core the routing loop is algebraically factored so u_hat [S, N*C]
is never materialized:
    iter0:   m0[(nc)]   = xsum @ W             (xsum = col-sum of x)
    V[d,n]   = sum_c W[d,(n c)] * mnorm[n,c]   (Wt-chunk @ Mblk, ap=16)
    b[s,n]   = x @ V                           (xT-chunk @ V,     ap=16)
    c        = softmax_n(b)
    G[n,d]   = c^T @ x                         (x-chunk @ c,      ap=16)
    m[(nc)]  = diag_n(W^T G)                   (W-chunk @ G^T,    ap=16)
    squash: inv = rsqrt(|m|^2) on DVE          (bit trick + 1 Newton step)
All routing matmuls keep the tiny capsule dim (16) as the moving side, so
PE streaming cost is ~16 cycles/matmul; the only large PE work is the
x-transposes (needed for the d-major contraction in b = x @ V).
"""
import numpy as np
import concourse.bass as bass
import concourse.mybir as mybir
import concourse.tile as tile
from concourse import bacc
from concourse.bass import ts, ds
from concourse.bass_utils import run_bass_kernel_spmd
from concourse.tile import add_dep_helper

F32 = mybir.dt.float32
U32 = mybir.dt.uint32
BF16 = mybir.dt.bfloat16
AF = mybir.ActivationFunctionType
AX = mybir.AxisListType
OP = mybir.AluOpType

NCORES = 8
B, S, D = 32, 512, 768
N, C = 16, 32
NC = N * C            # 512
BL = B // NCORES      # 4 batches per core
EPS = 1e-7
SCN = S // 128        # 4 s-chunks
DCN = D // 128        # 6 d-chunks
KCN = NC // 128       # 4 nc-chunks
ROUTINGS = 3

# const tile column layout (all bf16)
CID = 0               # [128, 128] identity (PE transposes)
CMASK = 128           # [128, 256] diag mask[(nl,c), (b,kc,n)] = (n == 4*kc+nl)
CSEL = 384            # [128, 4]   sel[p, j] = (p//32 == j)
CONE = 388            # [128, 1]   ones
CBS = 392             # [128, 128] bsel[k, m] = (k//32 == m//32)
CONW = 520


def _build_module():
    nc = bacc.Bacc("TRN2", target_bir_lowering=False, num_devices=NCORES)
    X = nc.dram_tensor("x", [BL, S, D], BF16, kind="ExternalInput")
    W = nc.dram_tensor("w", [D, NC], BF16, kind="ExternalInput")
    CON = nc.dram_tensor("consts", [128, CONW], BF16, kind="ExternalInput")
    OUT = nc.dram_tensor("out", [BL, N, C], F32, kind="ExternalOutput")

    with tile.TileContext(nc) as tc:
        with (
            tc.tile_pool(name="const", bufs=1) as pc,
            tc.tile_pool(name="rt", bufs=2) as prt,
            tc.tile_pool(name="pmm", bufs=1, space="PSUM") as pmm,
            tc.tile_pool(name="ptr", bufs=5, space="PSUM") as ptr,
        ):
            def cpd(dst, src):
                return nc.vector.tensor_copy(dst, src)

            def cpa(dst, src):
                return nc.scalar.copy(dst, src)

            # ---- persistent tiles ----
            con = pc.tile([128, CONW], BF16, tag="con")
            wsb = pc.tile([128, DCN, NC], BF16, tag="w")
            wtsb = pc.tile([128, KCN, D], BF16, tag="wt")
            xsumb = pc.tile([128, BL * DCN], BF16, tag="xsum")
            epst = pc.tile([128, 1], F32, tag="eps")
            nc.vector.memset(epst[:], EPS)
            magict = pc.tile([128, 16], U32, tag="magic")
            nc.vector.memset(magict[:], 0x5F3759DF)

            def prefetch_act(func):
                # dummy [1,1] activation hoists the ACT table load early
                dum = prt.tile([1, 1], F32, tag="dum")
                nc.scalar.activation(dum[:], epst[0:1, :], func)

            # consts ride the ACT queue; x batches + W ride the SP queue.
            # W sits between x1 and x2 so WT transposes clear the PE early;
            # the last batch arrives in s-chunks so stage A can track it.
            nc.scalar.dma_start(con[:], CON[:, :])
            xbs = [pc.tile([128, SCN, D], BF16, tag=f"xb{b}", name=f"xb_{b}")
                   for b in range(BL)]
            nc.sync.dma_start(
                xbs[0][:], X[0, :, :].rearrange("(sc p) d -> p sc d", p=128)
            )
            nc.sync.dma_start(
                wsb[:], W[:, :].rearrange("(dc p) n -> p dc n", p=128)
            )
            for b in (1, 2):
                nc.sync.dma_start(
                    xbs[b][:],
                    X[b, :, :].rearrange("(sc p) d -> p sc d", p=128),
                )
            for sc in range(SCN):
                nc.sync.dma_start(
                    xbs[3][:, sc, :], X[3, ds(sc * 128, 128), :]
                )
            prefetch_act(AF.Exp)

            ident = con[:, CID:CID + 128]

            # ---- stage A: xT + xsum per batch; WT between b2 and b3 so
            # the W transposes fill the DMA wait for the last batch ----
            pxs = pmm.tile([128, BL * DCN], F32, tag="seq")
            xts = [pc.tile([128, DCN, S], BF16, tag=f"xt{b}", name=f"xt_{b}")
                   for b in range(BL)]

            def stage_a(b):
                xb = xbs[b]
                xt = xts[b]
                for j in range(DCN // 2):
                    pxt = ptr.tile([128, 2 * S], BF16, tag="tr")
                    for h in range(2):
                        dc = 2 * j + h
                        for sc in range(SCN):
                            nc.tensor.transpose(
                                pxt[:, ds(h * S + sc * 128, 128)],
                                xb[:, sc, ds(dc * 128, 128)],
                                ident,
                            )
                        for sc in range(SCN):
                            nc.tensor.matmul(
                                pxs[:, ds(b * DCN + dc, 1)],
                                xb[:, sc, ds(dc * 128, 128)],
                                con[:, CONE:CONE + 1],
                                start=(sc == 0),
                                stop=(sc == SCN - 1),
                            )
                    eng = cpa if j == 1 else cpd
                    eng(xt[:, 2 * j:2 * j + 2, :].rearrange(
                        "p a b -> p (a b)"), pxt[:])
                cpd(xsumb[:, ds(b * DCN, DCN)], pxs[:, ds(b * DCN, DCN)])

            def stage_a3():
                # last batch: transposes emitted s-chunk-major (chunks arrive
                # via four DMAs), xsum evacuated before the pair tiles so
                # iter0 can start the moment the last chunk lands
                xb, xt = xbs[3], xts[3]
                p3 = [ptr.tile([128, 2 * S], BF16, tag="tr",
                               name=f"tr3_{j}") for j in range(3)]
                for sc in range(SCN):
                    for dc in range(DCN):
                        nc.tensor.transpose(
                            p3[dc // 2][:, ds((dc % 2) * S + sc * 128, 128)],
                            xb[:, sc, ds(dc * 128, 128)],
                            ident,
                        )
                for dc in range(DCN):
                    for sc in range(SCN):
                        nc.tensor.matmul(
                            pxs[:, ds(3 * DCN + dc, 1)],
                            xb[:, sc, ds(dc * 128, 128)],
                            con[:, CONE:CONE + 1],
                            start=(sc == 0),
                            stop=(sc == SCN - 1),
                        )
                xi = cpd(xsumb[:, ds(3 * DCN, DCN)],
                         pxs[:, ds(3 * DCN, DCN)])
                ji = cpa(xt[:, 2:4, :].rearrange("p a b -> p (a b)"),
                         p3[1][:])
                add_dep_helper(ji.ins, xi.ins, sync=False,
                               reason="xsum3 first")
                return xi, p3

            for b in range(2):
                stage_a(b)
            for kc in range(KCN):
                ptw = ptr.tile([128, 2 * S], BF16, tag="tr")
                for dc in range(DCN):
                    nc.tensor.transpose(
                        ptw[:, ts(dc, 128)],
                        wsb[:, dc, ds(kc * 128, 128)],
                        ident,
                    )
                (cpd if kc % 2 == 0 else cpa)(wtsb[:, kc, :], ptw[:, 0:768])
            stage_a(2)
            xi3, p3 = stage_a3()

            # ---- routing ----
            maskr = con[:, CMASK:CMASK + BL * KCN * N]

            def squash(pot, src_cols, it):
                """pot: psum [128, (b kc[ n])] -> returns mnorm tile.
                src_cols=1 for iter0 (pot is [128, (b kc)] = m directly)."""
                small = prt.tile([128, 16], BF16, tag="m", name=f"m{it}")
                if src_cols == 1:
                    nc.vector.tensor_copy(small[:], pot[:])
                    m = small
                    skip_sq = False
                else:
                    pm = prt.tile([128, BL * KCN * N], BF16, tag="pm")
                    nc.vector.tensor_mul(pm[:], pot[:], maskr)
                    with nc.allow_low_precision("single nonzero per group"):
                        nc.vector.tensor_reduce(
                            small[:],
                            pm[:].rearrange("p (g n) -> p g n", g=BL * KCN),
                            axis=AX.X,
                            op=OP.add,
                        )
                    m = small
                    skip_sq = False
                if not skip_sq:
                    sq = prt.tile([128, 16], BF16, tag="sq", name=f"sq{it}")
                    nc.vector.tensor_mul(sq[:], m[:], m[:])

                # fused capsule-group sum + broadcast: bsel[k,m]=(k//32==m//32)
                pnsq = pmm.tile([128, 16], F32, tag="seq", name=f"nsq{it}")
                nc.tensor.matmul(
                    pnsq[:],
                    con[:, CBS:CBS + 128],
                    sq[:],
                    start=True,
                    stop=True,
                )
                # rsqrt on DVE only (bit trick + 1 Newton step); keeps the
                # ACT table pinned to Exp for the whole kernel. nsq is
                # O(10..100) here so the reference's +eps is a no-op in bf16.
                y0u = prt.tile([128, 16], U32, tag="y0u", name=f"y0u{it}")
                nc.vector.tensor_scalar(
                    y0u[:], pnsq[:].bitcast(U32), 1, None,
                    OP.logical_shift_right,
                )
                nc.vector.tensor_sub(y0u[:], magict[:], y0u[:])
                y0f = y0u[:].bitcast(F32)
                if it == ROUTINGS - 1:
                    # full-precision normalize for the actual output
                    t1 = prt.tile([128, 16], F32, tag="nt1",
                                  name=f"nt1{it}")
                    nc.vector.tensor_mul(t1[:], pnsq[:], y0f)
                    nc.vector.tensor_mul(t1[:], t1[:], y0f)
                    nc.vector.tensor_scalar(t1[:], t1[:], -0.5, 1.5,
                                            OP.mult, OP.add)
                    pinv = prt.tile([128, 16], F32, tag="rsq",
                                    name=f"rsq{it}")
                    nc.vector.tensor_mul(pinv[:], y0f, t1[:])
                else:
                    pinv = y0u[:].bitcast(F32)
                if it == ROUTINGS - 1:
                    mnorm = prt.tile([128, 16], BF16, tag="mnf", name="mnf")
                    nc.vector.tensor_mul(mnorm[:], m[:], pinv[:])
                    return mnorm, None
                # pre-scatter m over the diag mask on the idle GPSIMD engine
                # while the DVE rsqrt chain runs; fold pinv in afterwards
                mm1 = prt.tile([128, BL * KCN * N], BF16, tag="mm1",
                               name=f"mm1_{it}")
                m_bc = bass.AP(
                    tensor=m.tensor,
                    offset=m.offset,
                    ap=[m.ap[0], [1, BL * KCN], [0, N]],
                )
                nc.gpsimd.tensor_mul(
                    mm1[:].rearrange("p (g n) -> p g n", g=BL * KCN),
                    m_bc,
                    maskr.rearrange("p (g n) -> p g n", g=BL * KCN),
                )
                mblk = prt.tile([128, BL * KCN * N], BF16, tag="mblk",
                                name=f"mblk{it}")
                pv_ap = pinv if isinstance(pinv, bass.AP) else pinv[:]
                pi_bc = bass.AP(
                    tensor=pv_ap.tensor,
                    offset=pv_ap.offset,
                    ap=[pv_ap.ap[0], [1, BL * KCN], [0, N]],
                )
                mbi = nc.vector.tensor_mul(
                    mblk[:].rearrange("p (g n) -> p g n", g=BL * KCN),
                    mm1[:].rearrange("p (g n) -> p g n", g=BL * KCN),
                    pi_bc,
                )
                return mblk, mbi

            def v_and_b(mblk, it):
                """V -> b (psum), half-batch interleaved with fully
                independent tiles per half so the evacuations overlap the
                other half's matmuls."""
                HB = BL * DCN * N // 2
                vsbs = []
                for half in range(2):
                    pv = pmm.tile([128, HB], F32, tag=f"big{half}",
                                  name=f"pv{it}_{half}")
                    for b in (0, 1) if half == 0 else (2, 3):
                        for dc in range(DCN):
                            col = ((b % 2) * DCN + dc) * N
                            for kc in range(KCN):
                                nc.tensor.matmul(
                                    pv[:, ds(col, N)],
                                    wtsb[:, kc, ds(dc * 128, 128)],
                                    mblk[:, ds((b * KCN + kc) * N, N)],
                                    start=(kc == 0),
                                    stop=(kc == KCN - 1),
                                )
                    vsb = prt.tile([128, HB], BF16, tag=f"vsb{half}",
                                   name=f"vsb{it}_{half}")
                    (nc.scalar.copy if half == 0
                     else nc.vector.tensor_copy)(vsb[:], pv[:])
                    vsbs.append(vsb)
                pb = pmm.tile([128, BL * SCN * N], F32, tag="seq")
                for b in range(BL):
                    for sc in range(SCN):
                        for dc in range(DCN):
                            nc.tensor.matmul(
                                pb[:, ds((b * SCN + sc) * N, N)],
                                xts[b][:, dc, ds(sc * 128, 128)],
                                vsbs[b // 2][:, ds(((b % 2) * DCN + dc) * N, N)],
                                start=(dc == 0),
                                stop=(dc == DCN - 1),
                            )
                return pb

            # iter 0: uniform routing weights -> m0 = xsum @ W (diag
            # blocks). Everything through b1 is batch-independent, so
            # batches 0-2 run while batch 3's chain trails the last DMA.
            pb = pmm.tile([128, BL * SCN * N], F32, tag="seq")

            def iter0_group(bs, be, big):
                nb = be - bs
                potg = pmm.tile([128, nb * KCN], F32, tag=big,
                                name=f"pot0_{bs}")
                for b in range(bs, be):
                    for kc in range(KCN):
                        for dc in range(DCN):
                            nc.tensor.matmul(
                                potg[:, ds((b - bs) * KCN + kc, 1)],
                                wsb[:, dc, ds(kc * 128, 128)],
                                xsumb[:, ds(b * DCN + dc, 1)],
                                start=(dc == 0),
                                stop=(dc == DCN - 1),
                            )
                w16 = nb * KCN
                mg = prt.tile([128, w16], BF16, tag="m", name=f"m0_{bs}")
                nc.vector.tensor_copy(mg[:], potg[:])
                sqg = prt.tile([128, w16], BF16, tag="sq", name=f"sq0_{bs}")
                nc.vector.tensor_mul(sqg[:], mg[:], mg[:])
                png = ptr.tile([128, w16], F32, tag="tr", name=f"nsq0_{bs}")
                nc.tensor.matmul(png[:], con[:, CBS:CBS + 128], sqg[:],
                                 start=True, stop=True)
                y0u = prt.tile([128, w16], U32, tag="y0u", name=f"y0u0_{bs}")
                nc.vector.tensor_scalar(
                    y0u[:], png[:].bitcast(U32), 1, None,
                    OP.logical_shift_right,
                )
                nc.vector.tensor_sub(y0u[:], magict[:, 0:w16], y0u[:])
                # no Newton step here: iter-0 norms only steer routing
                # logits, where the bit-trick's ~3% error is tolerable
                y0f = y0u[:].bitcast(F32)
                w256 = nb * KCN * N
                mm1 = prt.tile([128, w256], BF16, tag="mm1",
                               name=f"mm10_{bs}")
                m_bc = bass.AP(
                    tensor=mg.tensor, offset=mg.offset,
                    ap=[mg.ap[0], [1, w16], [0, N]],
                )
                nc.gpsimd.tensor_mul(
                    mm1[:].rearrange("p (g n) -> p g n", g=w16),
                    m_bc,
                    maskr[:, ds(bs * KCN * N, w256)].rearrange(
                        "p (g n) -> p g n", g=w16),
                )
                mblkg = prt.tile([128, w256], BF16, tag="mblk",
                                 name=f"mblk0_{bs}")
                pi_bc = bass.AP(
                    tensor=y0f.tensor, offset=y0f.offset,
                    ap=[y0f.ap[0], [1, w16], [0, N]],
                )
                mbig = nc.vector.tensor_mul(
                    mblkg[:].rearrange("p (g n) -> p g n", g=w16),
                    mm1[:].rearrange("p (g n) -> p g n", g=w16),
                    pi_bc,
                )
                pv = pmm.tile([128, nb * DCN * N], F32, tag=big,
                              name=f"pv0_{bs}")
                for b in range(bs, be):
                    for dc in range(DCN):
                        col = ((b - bs) * DCN + dc) * N
                        for kc in range(KCN):
                            nc.tensor.matmul(
                                pv[:, ds(col, N)],
                                wtsb[:, kc, ds(dc * 128, 128)],
                                mblkg[:, ds(((b - bs) * KCN + kc) * N, N)],
                                start=(kc == 0),
                                stop=(kc == KCN - 1),
                            )
                vsb = prt.tile([128, nb * DCN * N], BF16, tag=f"vsb0{bs}",
                               name=f"vsb0_{bs}")
                (cpa if bs == 0 else cpd)(vsb[:], pv[:])
                return mbig, vsb

            mbiA, vsbA = iter0_group(0, 3, "big0")
            for b in range(3):
                for sc in range(SCN):
                    for dc in range(DCN):
                        nc.tensor.matmul(
                            pb[:, ds((b * SCN + sc) * N, N)],
                            xts[b][:, dc, ds(sc * 128, 128)],
                            vsbA[:, ds((b * DCN + dc) * N, N)],
                            start=(dc == 0),
                            stop=(dc == DCN - 1),
                        )
            mbiB, vsbB = iter0_group(3, 4, "big1")
            # deferred batch-3 xt evacuations: pinned after batch 3's squash
            # chain so they don't occupy the DVE/ACT queues ahead of it
            for j, eng in ((0, cpa), (2, cpd)):
                ei = eng(xts[3][:, 2 * j:2 * j + 2, :].rearrange(
                    "p a b -> p (a b)"), p3[j][:])
                add_dep_helper(ei.ins, mbiB.ins, sync=False,
                               reason="xt3 after squash0")
            for sc in range(SCN):
                for dc in range(DCN):
                    nc.tensor.matmul(
                        pb[:, ds((3 * SCN + sc) * N, N)],
                        xts[3][:, dc, ds(sc * 128, 128)],
                        vsbB[:, ds(dc * N, N)],
                        start=(dc == 0),
                        stop=(dc == DCN - 1),
                    )

            for it in range(1, ROUTINGS):
                # softmax over n
                expb = prt.tile([128, BL * SCN * N], BF16, tag="expb",
                                name=f"expb{it}")
                nc.scalar.activation(expb[:], pb[:], AF.Exp)
                zsum = prt.tile([128, BL * SCN], BF16, tag="zsum",
                                name=f"zsum{it}")
                with nc.allow_low_precision("softmax denom; scale cancels"):
                    nc.vector.tensor_reduce(
                        zsum[:],
                        expb[:].rearrange("p (g n) -> p g n", g=BL * SCN),
                        axis=AX.X,
                        op=OP.add,
                    )
                zrec = prt.tile([128, BL * SCN], F32, tag="zrec",
                                name=f"zrec{it}")
                nc.vector.reciprocal(zrec[:], zsum[:])
                cw = prt.tile([128, BL * SCN * N], BF16, tag="cw",
                              name=f"cw{it}")
                zr_bc = bass.AP(
                    tensor=zrec.tensor,
                    offset=zrec.offset,
                    ap=[zrec.ap[0], [1, BL * SCN], [0, N]],
                )
                nc.vector.tensor_mul(
                    cw[:].rearrange("p (g n) -> p g n", g=BL * SCN),
                    expb[:].rearrange("p (g n) -> p g n", g=BL * SCN),
                    zr_bc,
                )
                # G^T[d, n] per (b, dc), then outT[(nc), n] per (b, kc);
                # half-batch interleaved with independent tiles per half
                HB = BL * DCN * N // 2
                gsbs = []
                for half in range(2):
                    pg = pmm.tile([128, HB], F32, tag=f"big{half}",
                                  name=f"gp{it}_{half}")
                    for b in (0, 1) if half == 0 else (2, 3):
                        for dc in range(DCN):
                            col = ((b % 2) * DCN + dc) * N
                            for sc in range(SCN):
                                nc.tensor.matmul(
                                    pg[:, ds(col, N)],
                                    xbs[b][:, sc, ds(dc * 128, 128)],
                                    cw[:, ds((b * SCN + sc) * N, N)],
                                    start=(sc == 0),
                                    stop=(sc == SCN - 1),
                                )
                    gsb = prt.tile([128, HB], BF16, tag=f"gsb{half}",
                                   name=f"gsb{it}_{half}")
                    (nc.scalar.copy if half == 0
                     else nc.vector.tensor_copy)(gsb[:], pg[:])
                    gsbs.append(gsb)
                pot = pmm.tile([128, BL * KCN * N], F32, tag="seq",
                               name=f"potp{it}")
                for b in range(BL):
                    for kc in range(KCN):
                        for dc in range(DCN):
                            nc.tensor.matmul(
                                pot[:, ds((b * KCN + kc) * N, N)],
                                wsb[:, dc, ds(kc * 128, 128)],
                                gsbs[b // 2][:, ds(((b % 2) * DCN + dc) * N, N)],
                                start=(dc == 0),
                                stop=(dc == DCN - 1),
                            )
                mnorm, _ = squash(pot, N, it)
                if it < ROUTINGS - 1:
                    pb = v_and_b(mnorm, it)

            # final output: transpose to [(b kc), (nl c)] so each DMA
            # descriptor is a 512-byte contiguous DRAM run
            pfin = pmm.tile([16, 128], BF16, tag="seq")
            nc.tensor.transpose(pfin[:], mnorm[:], ident)
            fsb = prt.tile([16, 128], F32, tag="fsb")
            nc.scalar.copy(fsb[:], pfin[:])
            nc.sync.dma_start(
                OUT.rearrange("b (kc nl) c -> (b kc) (nl c)", kc=KCN, nl=4),
                fsb[:],
            )

    nc.compile()
    return nc


def _make_consts():
    import ml_dtypes
    con = np.zeros((128, CONW), dtype=np.float32)
    con[:, CID:CID + 128] = np.eye(128, dtype=np.float32)
    p = np.arange(128)
    for b in range(BL):
        for kc in range(KCN):
            for n in range(N):
                con[:, CMASK + (b * KCN + kc) * N + n] = (n == 4 * kc + p // 32)
    for j in range(4):
        con[:, CSEL + j] = (p // 32 == j)
    con[:, CONE] = 1.0
    con[:, CBS:CBS + 128] = (p[:, None] // 32 == p[None, :] // 32)
    return con.astype(ml_dtypes.bfloat16)


_NC_CACHE = []


def kernel(x: np.ndarray, W: np.ndarray) -> np.ndarray:
    import ml_dtypes
    assert x.shape == (B, S, D) and W.shape == (1, D, NC)
    if not _NC_CACHE:
        _NC_CACHE.append(_build_module())
    nc = _NC_CACHE[0]
    con = _make_consts()
    w2 = np.ascontiguousarray(W[0]).astype(ml_dtypes.bfloat16)
    xb = x.astype(ml_dtypes.bfloat16)
    in_maps = []
    for i in range(NCORES):
        m = {
            "x": np.ascontiguousarray(xb[i * BL:(i + 1) * BL]),
            "w": w2,
            "consts": con,
        }
        in_maps.append(m)
    res = run_bass_kernel_spmd(nc, in_maps, list(range(NCORES)))
    out = np.concatenate([res.results[i]["out"] for i in range(NCORES)], axis=0)
    return out.astype(np.float32)



# revision 26
# speedup vs baseline: 1.6136x; 1.0006x over previous
"""Trainium2 Bass kernel for the capsule-routing layer (nn_Caps_Layer).

Full inputs: x [32, 512, 768] f32, W [1, 768, 512] f32.
Output: [32, 16, 32] f32.

Strategy: data-parallel over batch across 8 NeuronCores (4 batches/core),
inputs converted to bf16 on the host (halves the HBM traffic; rel-err
budget 2e-2 >> bf16's ~5e-3).

Per core the routing loop is algebraically factored so u_hat [S, N*C]
is never materialized:
    iter0:   m0[(nc)]   = xsum @ W             (xsum = col-sum of x)
    V[d,n]   = sum_c W[d,(n c)] * mnorm[n,c]   (Wt-chunk @ Mblk, ap=16)
    b[s,n]   = x @ V                           (xT-chunk @ V,     ap=16)
    c        = softmax_n(b)
    G[n,d]   = c^T @ x                         (x-chunk @ c,      ap=16)
    m[(nc)]  = diag_n(W^T G)                   (W-chunk @ G^T,    ap=16)
    squash: inv = rsqrt(|m|^2) on DVE          (bit trick + 1 Newton step)
All routing matmuls keep the tiny capsule dim (16) as the moving side, so
PE streaming cost is ~16 cycles/matmul; the only large PE work is the
x-transposes (needed for the d-major contraction in b = x @ V).
"""
import numpy as np
import concourse.bass as bass
import concourse.mybir as mybir
import concourse.tile as tile
from concourse import bacc
from concourse.bass import ts, ds
from concourse.bass_utils import run_bass_kernel_spmd
from concourse.tile import add_dep_helper

F32 = mybir.dt.float32
U32 = mybir.dt.uint32
BF16 = mybir.dt.bfloat16
AF = mybir.ActivationFunctionType
AX = mybir.AxisListType
OP = mybir.AluOpType

NCORES = 8
B, S, D = 32, 512, 768
N, C = 16, 32
NC = N * C            # 512
BL = B // NCORES      # 4 batches per core
EPS = 1e-7
SCN = S // 128        # 4 s-chunks
DCN = D // 128        # 6 d-chunks
KCN = NC // 128       # 4 nc-chunks
ROUTINGS = 3

# const tile column layout (all bf16)
CID = 0               # [128, 128] identity (PE transposes)
CMASK = 128           # [128, 256] diag mask[(nl,c), (b,kc,n)] = (n == 4*kc+nl)
CSEL = 384            # [128, 4]   sel[p, j] = (p//32 == j)
CONE = 388            # [128, 1]   ones
CBS = 392             # [128, 128] bsel[k, m] = (k//32 == m//32)
CONW = 520


def _build_module():
    nc = bacc.Bacc("TRN2", target_bir_lowering=False, num_devices=NCORES)
    X = nc.dram_tensor("x", [BL, S, D], BF16, kind="ExternalInput")
    W = nc.dram_tensor("w", [D, NC], BF16, kind="ExternalInput")
    CON = nc.dram_tensor("consts", [128, CONW], BF16, kind="ExternalInput")
    OUT = nc.dram_tensor("out", [BL, N, C], F32, kind="ExternalOutput")

    with tile.TileContext(nc) as tc:
        with (
            tc.tile_pool(name="const", bufs=1) as pc,
            tc.tile_pool(name="rt", bufs=2) as prt,
            tc.tile_pool(name="pmm", bufs=1, space="PSUM") as pmm,
            tc.tile_pool(name="ptr", bufs=5, space="PSUM") as ptr,
        ):
            def cpd(dst, src):
                return nc.vector.tensor_copy(dst, src)

            def cpa(dst, src):
                return nc.scalar.copy(dst, src)

            # ---- persistent tiles ----
            con = pc.tile([128, CONW], BF16, tag="con")
            wsb = pc.tile([128, DCN, NC], BF16, tag="w")
            wtsb = pc.tile([128, KCN, D], BF16, tag="wt")
            xsumb = pc.tile([128, BL * DCN], BF16, tag="xsum")
            epst = pc.tile([128, 1], F32, tag="eps")
            nc.vector.memset(epst[:], EPS)
            magict = pc.tile([128, 16], U32, tag="magic")
            nc.vector.memset(magict[:], 0x5F3759DF)
            # f32 identity built on-device (no DMA): ones, then zero off-diag
            identf = pc.tile([128, 128], F32, tag="idf")
            nc.vector.memset(identf[:], 1.0)
            nc.gpsimd.affine_select(identf[:], identf[:], [[1, 128]],
                                    OP.is_equal, 0.0, base=0,
                                    channel_multiplier=-1)

            def prefetch_act(func):
                # dummy [1,1] activation hoists the ACT table load early
                dum = prt.tile([1, 1], F32, tag="dum")
                nc.scalar.activation(dum[:], epst[0:1, :], func)

            # consts ride the ACT queue; x batches + W ride the SP queue.
            # W sits between x1 and x2 so WT transposes clear the PE early;
            # the last batch arrives in s-chunks so stage A can track it.
            nc.scalar.dma_start(con[:], CON[:, :])
            xbs = [pc.tile([128, SCN, D], BF16, tag=f"xb{b}", name=f"xb_{b}")
                   for b in range(BL)]
            nc.sync.dma_start(
                xbs[0][:], X[0, :, :].rearrange("(sc p) d -> p sc d", p=128)
            )
            nc.sync.dma_start(
                wsb[:], W[:, :].rearrange("(dc p) n -> p dc n", p=128)
            )
            for b in (1, 2):
                nc.sync.dma_start(
                    xbs[b][:],
                    X[b, :, :].rearrange("(sc p) d -> p sc d", p=128),
                )
            for sc in range(SCN):
                nc.sync.dma_start(
                    xbs[3][:, sc, :], X[3, ds(sc * 128, 128), :]
                )
            prefetch_act(AF.Exp)

            ident = con[:, CID:CID + 128]

            # ---- stage A: xT + xsum per batch; WT between b2 and b3 so
            # the W transposes fill the DMA wait for the last batch ----
            pxs = pmm.tile([128, BL * DCN], F32, tag="seq")
            xts = [pc.tile([128, DCN, S], BF16, tag=f"xt{b}", name=f"xt_{b}")
                   for b in range(BL)]

            def stage_a(b):
                xb = xbs[b]
                xt = xts[b]
                for j in range(DCN // 2):
                    pxt = ptr.tile([128, 2 * S], BF16, tag="tr")
                    for h in range(2):
                        dc = 2 * j + h
                        for sc in range(SCN):
                            nc.tensor.transpose(
                                pxt[:, ds(h * S + sc * 128, 128)],
                                xb[:, sc, ds(dc * 128, 128)],
                                ident,
                            )
                        for sc in range(SCN):
                            nc.tensor.matmul(
                                pxs[:, ds(b * DCN + dc, 1)],
                                xb[:, sc, ds(dc * 128, 128)],
                                con[:, CONE:CONE + 1],
                                start=(sc == 0),
                                stop=(sc == SCN - 1),
                            )
                    eng = cpa if j == 1 else cpd
                    eng(xt[:, 2 * j:2 * j + 2, :].rearrange(
                        "p a b -> p (a b)"), pxt[:])
                cpd(xsumb[:, ds(b * DCN, DCN)], pxs[:, ds(b * DCN, DCN)])

            def stage_a3():
                # last batch: transposes emitted s-chunk-major (chunks arrive
                # via four DMAs), xsum evacuated before the pair tiles so
                # iter0 can start the moment the last chunk lands
                xb, xt = xbs[3], xts[3]
                p3 = [ptr.tile([128, 2 * S], BF16, tag="tr",
                               name=f"tr3_{j}") for j in range(3)]
                for sc in range(SCN):
                    for dc in range(DCN):
                        nc.tensor.transpose(
                            p3[dc // 2][:, ds((dc % 2) * S + sc * 128, 128)],
                            xb[:, sc, ds(dc * 128, 128)],
                            ident,
                        )
                for dc in range(DCN):
                    for sc in range(SCN):
                        nc.tensor.matmul(
                            pxs[:, ds(3 * DCN + dc, 1)],
                            xb[:, sc, ds(dc * 128, 128)],
                            con[:, CONE:CONE + 1],
                            start=(sc == 0),
                            stop=(sc == SCN - 1),
                        )
                xi = cpd(xsumb[:, ds(3 * DCN, DCN)],
                         pxs[:, ds(3 * DCN, DCN)])
                ji = cpa(xt[:, 2:4, :].rearrange("p a b -> p (a b)"),
                         p3[1][:])
                add_dep_helper(ji.ins, xi.ins, sync=False,
                               reason="xsum3 first")
                return xi, p3

            for b in range(2):
                stage_a(b)
            for kc in range(KCN):
                ptw = ptr.tile([128, 2 * S], BF16, tag="tr")
                for dc in range(DCN):
                    nc.tensor.transpose(
                        ptw[:, ts(dc, 128)],
                        wsb[:, dc, ds(kc * 128, 128)],
                        ident,
                    )
                (cpd if kc % 2 == 0 else cpa)(wtsb[:, kc, :], ptw[:, 0:768])
            stage_a(2)
            xi3, p3 = stage_a3()

            # ---- routing ----
            maskr = con[:, CMASK:CMASK + BL * KCN * N]

            def squash(pot, src_cols, it):
                """pot: psum [128, (b kc[ n])] -> returns mnorm tile.
                src_cols=1 for iter0 (pot is [128, (b kc)] = m directly)."""
                small = prt.tile([128, 16], BF16, tag="m", name=f"m{it}")
                if src_cols == 1:
                    nc.vector.tensor_copy(small[:], pot[:])
                    m = small
                    skip_sq = False
                else:
                    pm = prt.tile([128, BL * KCN * N], BF16, tag="pm")
                    nc.vector.tensor_mul(pm[:], pot[:], maskr)
                    with nc.allow_low_precision("single nonzero per group"):
                        nc.vector.tensor_reduce(
                            small[:],
                            pm[:].rearrange("p (g n) -> p g n", g=BL * KCN),
                            axis=AX.X,
                            op=OP.add,
                        )
                    m = small
                    skip_sq = False
                if not skip_sq:
                    sq = prt.tile([128, 16], BF16, tag="sq", name=f"sq{it}")
                    nc.vector.tensor_mul(sq[:], m[:], m[:])

                # fused capsule-group sum + broadcast: bsel[k,m]=(k//32==m//32)
                pnsq = pmm.tile([128, 16], F32, tag="seq", name=f"nsq{it}")
                nc.tensor.matmul(
                    pnsq[:],
                    con[:, CBS:CBS + 128],
                    sq[:],
                    start=True,
                    stop=True,
                )
                # rsqrt on DVE only (bit trick + 1 Newton step); keeps the
                # ACT table pinned to Exp for the whole kernel. nsq is
                # O(10..100) here so the reference's +eps is a no-op in bf16.
                y0u = prt.tile([128, 16], U32, tag="y0u", name=f"y0u{it}")
                nc.vector.tensor_scalar(
                    y0u[:], pnsq[:].bitcast(U32), 1, None,
                    OP.logical_shift_right,
                )
                nc.vector.tensor_sub(y0u[:], magict[:], y0u[:])
                y0f = y0u[:].bitcast(F32)
                if it == ROUTINGS - 1:
                    # full-precision normalize for the actual output
                    t1 = prt.tile([128, 16], F32, tag="nt1",
                                  name=f"nt1{it}")
                    nc.vector.tensor_mul(t1[:], pnsq[:], y0f)
                    nc.vector.tensor_mul(t1[:], t1[:], y0f)
                    nc.vector.tensor_scalar(t1[:], t1[:], -0.5, 1.5,
                                            OP.mult, OP.add)
                    pinv = prt.tile([128, 16], F32, tag="rsq",
                                    name=f"rsq{it}")
                    nc.vector.tensor_mul(pinv[:], y0f, t1[:])
                else:
                    pinv = y0u[:].bitcast(F32)
                if it == ROUTINGS - 1:
                    mnorm = prt.tile([128, 16], F32, tag="mnf", name="mnf")
                    nc.vector.tensor_mul(mnorm[:], m[:], pinv[:])
                    return mnorm, None
                # pre-scatter m over the diag mask on the idle GPSIMD engine
                # while the DVE rsqrt chain runs; fold pinv in afterwards
                mm1 = prt.tile([128, BL * KCN * N], BF16, tag="mm1",
                               name=f"mm1_{it}")
                m_bc = bass.AP(
                    tensor=m.tensor,
                    offset=m.offset,
                    ap=[m.ap[0], [1, BL * KCN], [0, N]],
                )
                nc.gpsimd.tensor_mul(
                    mm1[:].rearrange("p (g n) -> p g n", g=BL * KCN),
                    m_bc,
                    maskr.rearrange("p (g n) -> p g n", g=BL * KCN),
                )
                mblk = prt.tile([128, BL * KCN * N], BF16, tag="mblk",
                                name=f"mblk{it}")
                pv_ap = pinv if isinstance(pinv, bass.AP) else pinv[:]
                pi_bc = bass.AP(
                    tensor=pv_ap.tensor,
                    offset=pv_ap.offset,
                    ap=[pv_ap.ap[0], [1, BL * KCN], [0, N]],
                )
                mbi = nc.vector.tensor_mul(
                    mblk[:].rearrange("p (g n) -> p g n", g=BL * KCN),
                    mm1[:].rearrange("p (g n) -> p g n", g=BL * KCN),
                    pi_bc,
                )
                return mblk, mbi

            def v_and_b(mblk, it):
                """V -> b (psum), half-batch interleaved with fully
                independent tiles per half so the evacuations overlap the
                other half's matmuls."""
                HB = BL * DCN * N // 2
                vsbs = []
                for half in range(2):
                    pv = pmm.tile([128, HB], F32, tag=f"big{half}",
                                  name=f"pv{it}_{half}")
                    for b in (0, 1) if half == 0 else (2, 3):
                        for dc in range(DCN):
                            col = ((b % 2) * DCN + dc) * N
                            for kc in range(KCN):
                                nc.tensor.matmul(
                                    pv[:, ds(col, N)],
                                    wtsb[:, kc, ds(dc * 128, 128)],
                                    mblk[:, ds((b * KCN + kc) * N, N)],
                                    start=(kc == 0),
                                    stop=(kc == KCN - 1),
                                )
                    vsb = prt.tile([128, HB], BF16, tag=f"vsb{half}",
                                   name=f"vsb{it}_{half}")
                    (nc.scalar.copy if half == 0
                     else nc.vector.tensor_copy)(vsb[:], pv[:])
                    vsbs.append(vsb)
                pb = pmm.tile([128, BL * SCN * N], F32, tag="seq")
                for b in range(BL):
                    for sc in range(SCN):
                        for dc in range(DCN):
                            nc.tensor.matmul(
                                pb[:, ds((b * SCN + sc) * N, N)],
                                xts[b][:, dc, ds(sc * 128, 128)],
                                vsbs[b // 2][:, ds(((b % 2) * DCN + dc) * N, N)],
                                start=(dc == 0),
                                stop=(dc == DCN - 1),
                            )
                return pb

            # iter 0: uniform routing weights -> m0 = xsum @ W (diag
            # blocks). Everything through b1 is batch-independent, so
            # batches 0-2 run while batch 3's chain trails the last DMA.
            pb = pmm.tile([128, BL * SCN * N], F32, tag="seq")

            def iter0_group(bs, be, big):
                nb = be - bs
                potg = pmm.tile([128, nb * KCN], F32, tag=big,
                                name=f"pot0_{bs}")
                for b in range(bs, be):
                    for kc in range(KCN):
                        for dc in range(DCN):
                            nc.tensor.matmul(
                                potg[:, ds((b - bs) * KCN + kc, 1)],
                                wsb[:, dc, ds(kc * 128, 128)],
                                xsumb[:, ds(b * DCN + dc, 1)],
                                start=(dc == 0),
                                stop=(dc == DCN - 1),
                            )
                w16 = nb * KCN
                mg = prt.tile([128, w16], BF16, tag="m", name=f"m0_{bs}")
                nc.vector.tensor_copy(mg[:], potg[:])
                sqg = prt.tile([128, w16], BF16, tag="sq", name=f"sq0_{bs}")
                nc.vector.tensor_mul(sqg[:], mg[:], mg[:])
                png = ptr.tile([128, w16], F32, tag="tr", name=f"nsq0_{bs}")
                nc.tensor.matmul(png[:], con[:, CBS:CBS + 128], sqg[:],
                                 start=True, stop=True)
                y0u = prt.tile([128, w16], U32, tag="y0u", name=f"y0u0_{bs}")
                nc.vector.tensor_scalar(
                    y0u[:], png[:].bitcast(U32), 1, None,
                    OP.logical_shift_right,
                )
                nc.vector.tensor_sub(y0u[:], magict[:, 0:w16], y0u[:])
                # no Newton step here: iter-0 norms only steer routing
                # logits, where the bit-trick's ~3% error is tolerable
                y0f = y0u[:].bitcast(F32)
                w256 = nb * KCN * N
                mm1 = prt.tile([128, w256], BF16, tag="mm1",
                               name=f"mm10_{bs}")
                m_bc = bass.AP(
                    tensor=mg.tensor, offset=mg.offset,
                    ap=[mg.ap[0], [1, w16], [0, N]],
                )
                nc.gpsimd.tensor_mul(
                    mm1[:].rearrange("p (g n) -> p g n", g=w16),
                    m_bc,
                    maskr[:, ds(bs * KCN * N, w256)].rearrange(
                        "p (g n) -> p g n", g=w16),
                )
                mblkg = prt.tile([128, w256], BF16, tag="mblk",
                                 name=f"mblk0_{bs}")
                pi_bc = bass.AP(
                    tensor=y0f.tensor, offset=y0f.offset,
                    ap=[y0f.ap[0], [1, w16], [0, N]],
                )
                mbig = nc.vector.tensor_mul(
                    mblkg[:].rearrange("p (g n) -> p g n", g=w16),
                    mm1[:].rearrange("p (g n) -> p g n", g=w16),
                    pi_bc,
                )
                pv = pmm.tile([128, nb * DCN * N], F32, tag=big,
                              name=f"pv0_{bs}")
                for b in range(bs, be):
                    for dc in range(DCN):
                        col = ((b - bs) * DCN + dc) * N
                        for kc in range(KCN):
                            nc.tensor.matmul(
                                pv[:, ds(col, N)],
                                wtsb[:, kc, ds(dc * 128, 128)],
                                mblkg[:, ds(((b - bs) * KCN + kc) * N, N)],
                                start=(kc == 0),
                                stop=(kc == KCN - 1),
                            )
                vsb = prt.tile([128, nb * DCN * N], BF16, tag=f"vsb0{bs}",
                               name=f"vsb0_{bs}")
                (cpa if bs == 0 else cpd)(vsb[:], pv[:])
                return mbig, vsb

            mbiA, vsbA = iter0_group(0, 3, "big0")
            for b in range(3):
                for sc in range(SCN):
                    for dc in range(DCN):
                        nc.tensor.matmul(
                            pb[:, ds((b * SCN + sc) * N, N)],
                            xts[b][:, dc, ds(sc * 128, 128)],
                            vsbA[:, ds((b * DCN + dc) * N, N)],
                            start=(dc == 0),
                            stop=(dc == DCN - 1),
                        )
            mbiB, vsbB = iter0_group(3, 4, "big1")
            # deferred batch-3 xt evacuations: pinned after batch 3's squash
            # chain so they don't occupy the DVE/ACT queues ahead of it
            for j, eng in ((0, cpa), (2, cpd)):
                ei = eng(xts[3][:, 2 * j:2 * j + 2, :].rearrange(
                    "p a b -> p (a b)"), p3[j][:])
                add_dep_helper(ei.ins, mbiB.ins, sync=False,
                               reason="xt3 after squash0")
            for sc in range(SCN):
                for dc in range(DCN):
                    nc.tensor.matmul(
                        pb[:, ds((3 * SCN + sc) * N, N)],
                        xts[3][:, dc, ds(sc * 128, 128)],
                        vsbB[:, ds(dc * N, N)],
                        start=(dc == 0),
                        stop=(dc == DCN - 1),
                    )

            for it in range(1, ROUTINGS):
                # softmax over n
                expb = prt.tile([128, BL * SCN * N], BF16, tag="expb",
                                name=f"expb{it}")
                nc.scalar.activation(expb[:], pb[:], AF.Exp)
                zsum = prt.tile([128, BL * SCN], BF16, tag="zsum",
                                name=f"zsum{it}")
                with nc.allow_low_precision("softmax denom; scale cancels"):
                    nc.vector.tensor_reduce(
                        zsum[:],
                        expb[:].rearrange("p (g n) -> p g n", g=BL * SCN),
                        axis=AX.X,
                        op=OP.add,
                    )
                zrec = prt.tile([128, BL * SCN], F32, tag="zrec",
                                name=f"zrec{it}")
                nc.vector.reciprocal(zrec[:], zsum[:])
                cw = prt.tile([128, BL * SCN * N], BF16, tag="cw",
                              name=f"cw{it}")
                zr_bc = bass.AP(
                    tensor=zrec.tensor,
                    offset=zrec.offset,
                    ap=[zrec.ap[0], [1, BL * SCN], [0, N]],
                )
                nc.vector.tensor_mul(
                    cw[:].rearrange("p (g n) -> p g n", g=BL * SCN),
                    expb[:].rearrange("p (g n) -> p g n", g=BL * SCN),
                    zr_bc,
                )
                # G^T[d, n] per (b, dc), then outT[(nc), n] per (b, kc);
                # half-batch interleaved with independent tiles per half
                HB = BL * DCN * N // 2
                gsbs = []
                for half in range(2):
                    pg = pmm.tile([128, HB], F32, tag=f"big{half}",
                                  name=f"gp{it}_{half}")
                    for b in (0, 1) if half == 0 else (2, 3):
                        for dc in range(DCN):
                            col = ((b % 2) * DCN + dc) * N
                            for sc in range(SCN):
                                nc.tensor.matmul(
                                    pg[:, ds(col, N)],
                                    xbs[b][:, sc, ds(dc * 128, 128)],
                                    cw[:, ds((b * SCN + sc) * N, N)],
                                    start=(sc == 0),
                                    stop=(sc == SCN - 1),
                                )
                    gsb = prt.tile([128, HB], BF16, tag=f"gsb{half}",
                                   name=f"gsb{it}_{half}")
                    (nc.scalar.copy if half == 0
                     else nc.vector.tensor_copy)(gsb[:], pg[:])
                    gsbs.append(gsb)
                pot = pmm.tile([128, BL * KCN * N], F32, tag="seq",
                               name=f"potp{it}")
                for b in range(BL):
                    for kc in range(KCN):
                        for dc in range(DCN):
                            nc.tensor.matmul(
                                pot[:, ds((b * KCN + kc) * N, N)],
                                wsb[:, dc, ds(kc * 128, 128)],
                                gsbs[b // 2][:, ds(((b % 2) * DCN + dc) * N, N)],
                                start=(dc == 0),
                                stop=(dc == DCN - 1),
                            )
                mnorm, _ = squash(pot, N, it)
                if it < ROUTINGS - 1:
                    pb = v_and_b(mnorm, it)

            # final output: transpose to [(b kc), (nl c)] so each DMA
            # descriptor is a 512-byte contiguous DRAM run; DMA straight
            # from PSUM to skip the SBUF bounce on the tail
            pfin = pmm.tile([16, 128], F32, tag="seq")
            nc.tensor.transpose(pfin[:], mnorm[:], identf[:])
            fsb = prt.tile([16, 128], F32, tag="fsb")
            nc.vector.tensor_copy(fsb[:], pfin[:])
            nc.sync.dma_start(
                OUT.rearrange("b (kc nl) c -> (b kc) (nl c)", kc=KCN, nl=4),
                fsb[:],
            )

    nc.compile()
    return nc


def _make_consts():
    import ml_dtypes
    con = np.zeros((128, CONW), dtype=np.float32)
    con[:, CID:CID + 128] = np.eye(128, dtype=np.float32)
    p = np.arange(128)
    for b in range(BL):
        for kc in range(KCN):
            for n in range(N):
                con[:, CMASK + (b * KCN + kc) * N + n] = (n == 4 * kc + p // 32)
    for j in range(4):
        con[:, CSEL + j] = (p // 32 == j)
    con[:, CONE] = 1.0
    con[:, CBS:CBS + 128] = (p[:, None] // 32 == p[None, :] // 32)
    return con.astype(ml_dtypes.bfloat16)


_NC_CACHE = []


def kernel(x: np.ndarray, W: np.ndarray) -> np.ndarray:
    import ml_dtypes
    assert x.shape == (B, S, D) and W.shape == (1, D, NC)
    if not _NC_CACHE:
        _NC_CACHE.append(_build_module())
    nc = _NC_CACHE[0]
    con = _make_consts()
    w2 = np.ascontiguousarray(W[0]).astype(ml_dtypes.bfloat16)
    xb = x.astype(ml_dtypes.bfloat16)
    in_maps = []
    for i in range(NCORES):
        m = {
            "x": np.ascontiguousarray(xb[i * BL:(i + 1) * BL]),
            "w": w2,
            "consts": con,
        }
        in_maps.append(m)
    res = run_bass_kernel_spmd(nc, in_maps, list(range(NCORES)))
    out = np.concatenate([res.results[i]["out"] for i in range(NCORES)], axis=0)
    return out.astype(np.float32)

